# revision 17
# baseline (speedup 1.0000x reference)
"""Trainium2 Bass kernel for the CustomLSTM problem.

Strategy (data-parallel over batch, 8 cores):
  - Each core gets 8 of the 64 batch rows; weights are replicated.
  - Phase A (per core): xeT = W_emb @ x.T (transposed embed), then
    gx = xe @ W_ih_p.T + bias_total (input contribution to all gates for
    every timestep), written to DRAM as [T, B_l, 4H].
  - Scan (per core): for each t: gates = gx_t + h @ W_hh_p.T via PSUM
    accumulation (gx_t injected with an identity-weighted matmul),
    sigma/tanh on ScalarE, cell/hidden update on VectorE, h transposed
    back for the next step's stationary operand with PE transposes.
  - Gate columns are permuted host-side into per-hidden-slice chunks
    [i f o g] x 128 so each 512-column PSUM bank holds a complete slice.

The full outputs are reassembled host-side: outputs [B,T,H], h_T, c_T.
"""

import numpy as np

B, T, R, E, H = 64, 512, 256, 512, 1024
G4 = 4 * H
NCORES = 8
BL = B // NCORES  # batch rows per core


def _make_tc_class(tile, mybir, ScopedClock):
    class TC(tile.TileContext):
        """TileContext whose final drain splits multi-sem waits across
        single-wait nops (this walrus build allows 1 sync wait per CTRL)."""

        def _drain_and_barrier(self, tick_clock, wait_clock):
            nc = self.nc
            drain_inst = nc.sync.drain()
            wait_clock.add_sem_waits(
                drain_inst.ins, ScopedClock({None: tick_clock.global_clock})
            )
            si = drain_inst.ins.sync_info
            waits = list(si.on_wait or [])
            if len(waits) > 1:
                si.on_wait = waits[:1]
                for w in waits[1:]:
                    nop = nc.sync.nop(nofuse=True)
                    nsi = nop.ins.sync_info
                    if nsi is None:
                        nop.ins.sync_info = mybir.SyncInfo(on_wait=[w], on_update=[])
                    else:
                        nsi.on_wait = [w]
            nc.all_engine_barrier()
            popped = nc._tile_sem_poison_stack.pop()
            assert popped is self._sem_poison
            nc.clear_and_free_semaphores(list(self.sems.allocated().values()))
            nc.all_engine_barrier()

    return TC


def _split_multi_waits(nc, mybir, limit=1):
    """This walrus build rejects instructions carrying more than `limit`
    semaphore waits. Move extra waits onto fresh same-engine nops placed
    immediately before the instruction (engine queues execute in order, so
    semantics are preserved)."""
    n_new = 0
    for bb in nc.main_func.blocks:
        insts = bb.instructions
        out = []
        for inst in insts:
            si = inst.sync_info
            waits = list(si.on_wait) if si and si.on_wait else []
            if len(waits) > limit:
                extra, keep = waits[:-limit], waits[-limit:]
                for j in range(0, len(extra), limit):
                    chunk = extra[j : j + limit]
                    nop = mybir.InstNoOp(
                        name=f"{inst.name}-sw{j}",
                        sync_info=mybir.SyncInfo(on_wait=chunk, on_update=[]),
                        bass_nofuse=True,
                        engine=inst.engine,
                    )
                    out.append(nop)
                    n_new += 1
                si.on_wait = keep
            out.append(inst)
        if len(out) != len(insts):
            insts[:] = out
    return n_new


def _phase_a(nc, tc, bass, mybir, eye, x_d, wembT_d, wihT_d, biasg_d, gx_d,
             rows, t_steps):
    f32 = mybir.dt.float32
    NKB = G4 // 512
    RH = rows // 2  # phase-A half rows

    with tc.tile_pool(name="pa_w", bufs=1) as pa_w:
        wembT_sb = pa_w.tile([128, R // 128, E], f32)  # [p, kr, e]
        nc.sync.dma_start(
            wembT_sb, wembT_d.ap().rearrange("(k p) e -> p k e", p=128)
        )
        wihT_sb = pa_w.tile([128, E // 128, G4], f32)  # [p, ke, n]
        nc.sync.dma_start(
            wihT_sb, wihT_d.ap().rearrange("(k p) n -> p k n", p=128)
        )
        bias_bc = pa_w.tile([128, G4], f32)
        bias_src = bass.AP(
            tensor=biasg_d.ap().tensor,
            offset=0,
            ap=[[0, 128], [1, G4]],
        )
        nc.sync.dma_start(bias_bc, bias_src)

        for half in range(2):
            r0 = half * RH
            with (
                tc.tile_pool(name="pa_sb", bufs=1) as pa_sb,
                tc.tile_pool(name="pa_stage", bufs=4) as pa_stage,
                tc.tile_pool(name="pa_psT", bufs=2, space="PSUM") as pa_psT,
                tc.tile_pool(name="pa_ps", bufs=3, space="PSUM") as pa_ps,
                tc.tile_pool(name="pa_ps2", bufs=3, space="PSUM") as pa_ps2,
            ):
                # x.T for this half: [R, RH] as [128, R//128, RH]
                xT_sb = pa_sb.tile([128, R // 128, RH], f32)
                nrc = RH // 128
                for rc in range(nrc):
                    x_tile = pa_stage.tile([128, R], f32)
                    nc.sync.dma_start(
                        x_tile, x_d.ap()[r0 + rc * 128 : r0 + (rc + 1) * 128, :]
                    )
                    for k in range(R // 128):
                        pt = pa_psT.tile([128, 128], f32)
                        nc.tensor.transpose(
                            pt, x_tile[:, k * 128 : (k + 1) * 128], eye
                        )
                        nc.scalar.copy(
                            xT_sb[:, k, rc * 128 : (rc + 1) * 128], pt
                        )

                # A1: xeT[e, m] for this half: [E, RH] as [128, E//128, RH]
                xeT_sb = pa_sb.tile([128, E // 128, RH], f32)
                for mc in range(RH // 512):
                    for ec in range(E // 128):
                        ps = pa_ps.tile([128, 512], f32)
                        for k in range(R // 128):
                            nc.tensor.matmul(
                                ps,
                                lhsT=wembT_sb[:, k, ec * 128 : (ec + 1) * 128],
                                rhs=xT_sb[:, k, mc * 512 : (mc + 1) * 512],
                                start=(k == 0),
                                stop=(k == R // 128 - 1),
                            )
                        nc.scalar.copy(
                            xeT_sb[:, ec, mc * 512 : (mc + 1) * 512], ps
                        )

                # A2: gx[m, n] = xe @ W_ih_p.T + bias  (rows are b*T + t)
                gx_view = gx_d.ap().rearrange("(t b) n -> t b n", b=BL)
                for rc in range(nrc):
                    m_global = r0 + rc * 128
                    b_idx = m_global // t_steps
                    t0 = m_global % t_steps
                    for nb in range(NKB):
                        ps = pa_ps2.tile([128, 512], f32)
                        for k in range(E // 128):
                            nc.tensor.matmul(
                                ps,
                                lhsT=xeT_sb[:, k, rc * 128 : (rc + 1) * 128],
                                rhs=wihT_sb[:, k, nb * 512 : (nb + 1) * 512],
                                start=(k == 0),
                                stop=(k == E // 128 - 1),
                            )
                        gx_tile = pa_stage.tile([128, 512], f32)
                        nc.vector.tensor_add(
                            gx_tile, ps, bias_bc[:, nb * 512 : (nb + 1) * 512]
                        )
                        nc.sync.dma_start(
                            gx_view[
                                t0 : t0 + 128, b_idx, nb * 512 : (nb + 1) * 512
                            ],
                            gx_tile,
                        )


def build_module(t_steps=T, loop_step=4, use_loop=True, phase_a=True,
                 split_waits=True, scan_mode="v1", ngroups=4):
    """Build the Bass module. Returns nc.

    phase_a=False: gx is taken as an ExternalInput (for testing the scan).
    scan_mode: "v1" plain M=8 matmuls; "v2" column-tiled concurrent matmuls
    (ngroups PE column groups; the recurrent weight stream runs ngroups-way
    concurrent). v2 stores y in a grouped layout (assemble with grouped=True).
    """
    from contextlib import ExitStack

    import concourse.bass as bass
    import concourse.mybir as mybir
    import concourse.tile as tile
    from concourse.bass import ds
    from concourse.masks import make_identity
    from concourse.vector_clock import ScopedClock

    f32 = mybir.dt.float32
    AF = mybir.ActivationFunctionType
    TC = _make_tc_class(tile, mybir, ScopedClock)

    nc = bass.Bass("TRN2", target_bir_lowering=False, debug=False)

    rows = BL * t_steps  # x rows per core

    wembT_d = nc.dram_tensor("wembT", [R, E], f32, kind="ExternalInput")
    wihT_d = nc.dram_tensor("wihT", [E, G4], f32, kind="ExternalInput")
    whhT_d = nc.dram_tensor("whhT", [H, G4], f32, kind="ExternalInput")
    biasg_d = nc.dram_tensor("biasg", [1, G4], f32, kind="ExternalInput")

    y_d = nc.dram_tensor("y", [t_steps * BL, H], f32, kind="ExternalOutput")
    c_out_d = nc.dram_tensor("c_out", [BL, H], f32, kind="ExternalOutput")

    NKB = G4 // 512  # 8 gate banks (512 cols each, layout [i f o g] x 128)
    KH = H // 128    # 8 contraction chunks for the recurrent matmul
    GP = 8 * NKB     # 64 partition-rows per step in the packed gx layout

    if phase_a:
        x_d = nc.dram_tensor("x", [rows, R], f32, kind="ExternalInput")
        gx_d = nc.dram_tensor("gx", [t_steps * BL, G4], f32)  # internal
    else:
        x_d = None
        gx_d = nc.dram_tensor("gx", [t_steps * BL, G4], f32, kind="ExternalInput")

    with TC(nc) as tc, ExitStack() as stack:
        consts = stack.enter_context(tc.tile_pool(name="consts", bufs=1))
        eye = consts.tile([128, 128], f32)
        make_identity(nc, eye)

        if phase_a:
            _phase_a(nc, tc, bass, mybir, eye, x_d, wembT_d, wihT_d, biasg_d,
                     gx_d, rows, t_steps)

        # ---------------- Scan ----------------
        with (
            tc.tile_pool(name="sc_w", bufs=1) as sc_w,
            tc.tile_pool(name="sc_state", bufs=1) as sc_state,
            tc.tile_pool(name="sc_gx", bufs=2) as sc_gx,
            tc.tile_pool(name="sc_act", bufs=4) as sc_act,
            tc.tile_pool(name="sc_tmp", bufs=8) as sc_tmp,
            tc.tile_pool(name="sc_h", bufs=2) as sc_h,
            tc.tile_pool(name="sc_hg", bufs=2) as sc_hg,
            tc.tile_pool(name="sc_ps", bufs=4, space="PSUM") as sc_ps,
            tc.tile_pool(name="sc_psT", bufs=2, space="PSUM") as sc_psT,
        ):
            whhT_sb = sc_w.tile([128, KH, G4], f32)
            nc.sync.dma_start(
                whhT_sb, whhT_d.ap().rearrange("(k p) n -> p k n", p=128)
            )

            hT_stride = BL if scan_mode == "v1" else 32
            hT_a = sc_state.tile([128, hT_stride * KH], f32)
            hT_b = sc_state.tile([128, hT_stride * KH], f32)
            nc.vector.memset(hT_a, 0.0)
            if scan_mode != "v1":
                nc.vector.memset(hT_b, 0.0)

            eye8 = eye[0:BL, 0:BL]

            # bank -> (group, wave) mapping for the column-tiled scan
            per = (NKB + ngroups - 1) // ngroups
            grp_banks = [list(range(j * per, min((j + 1) * per, NKB)))
                         for j in range(ngroups)]

            if scan_mode == "v1":
                c_sb = sc_state.tile([BL, H], f32)
                nc.vector.memset(c_sb, 0.0)
            else:
                c_g = sc_state.tile([128, per * 128], f32)
                nc.vector.memset(c_g, 0.0)

            def step_v1(iv, hT_cur, hT_next):
                gxt = sc_gx.tile([BL, G4], f32)
                nc.sync.dma_start(gxt, gx_d.ap()[ds(iv * BL, BL), :])
                h_sb = sc_h.tile([BL, H], f32)
                for nb in range(NKB):
                    ps = sc_ps.tile([BL, 512], f32)
                    nc.tensor.matmul(
                        ps,
                        lhsT=eye8,
                        rhs=gxt[:, nb * 512 : (nb + 1) * 512],
                        start=True,
                        stop=False,
                    )
                    for k in range(KH):
                        nc.tensor.matmul(
                            ps,
                            lhsT=hT_cur[:, k * BL : (k + 1) * BL],
                            rhs=whhT_sb[:, k, nb * 512 : (nb + 1) * 512],
                            start=False,
                            stop=(k == KH - 1),
                        )
                    acts = sc_act.tile([BL, 512], f32)
                    nc.scalar.activation(acts[:, 0:384], ps[:, 0:384], AF.Sigmoid)
                    nc.scalar.activation(acts[:, 384:512], ps[:, 384:512], AF.Tanh)
                    cs = c_sb[:, nb * 128 : (nb + 1) * 128]
                    ig = sc_tmp.tile([BL, 128], f32)
                    nc.vector.tensor_mul(ig, acts[:, 0:128], acts[:, 384:512])
                    fc = sc_tmp.tile([BL, 128], f32)
                    nc.vector.tensor_mul(fc, acts[:, 128:256], cs)
                    nc.vector.tensor_add(cs, fc, ig)
                    tanh_c = sc_tmp.tile([BL, 128], f32)
                    nc.scalar.activation(tanh_c, cs, AF.Tanh)
                    hs = h_sb[:, nb * 128 : (nb + 1) * 128]
                    nc.vector.tensor_mul(hs, acts[:, 256:384], tanh_c)
                    # transpose h slice for next step's stationary operand
                    pT = sc_psT.tile([128, BL], f32)
                    nc.tensor.transpose(pT, hs, eye8)
                    nc.vector.tensor_copy(hT_next[:, nb * BL : (nb + 1) * BL], pT)
                nc.sync.dma_start(y_d.ap()[ds(iv * BL, BL), :], h_sb)

            def step_v2(iv, hT_cur, hT_next):
                gxt = sc_gx.tile([BL, G4], f32)
                nc.sync.dma_start(gxt, gx_d.ap()[ds(iv * BL, BL), :])
                h_sb = sc_h.tile([BL, H], f32)
                h_g = sc_hg.tile([128, per * 128], f32)
                for w in range(per):
                    gactive = [j for j in range(ngroups) if w < len(grp_banks[j])]
                    ga = len(gactive)
                    ps = sc_ps.tile([128, 512], f32)
                    for j in gactive:
                        nb = grp_banks[j][w]
                        nc.tensor.matmul(
                            ps[32 * j : 32 * (j + 1), :],
                            lhsT=eye[0:BL, 0:32],
                            rhs=gxt[:, nb * 512 : (nb + 1) * 512],
                            start=True,
                            stop=False,
                            tile_position=(0, 32 * j),
                        )
                        for k in range(KH):
                            nc.tensor.matmul(
                                ps[32 * j : 32 * (j + 1), :],
                                lhsT=hT_cur[:, k * 32 : k * 32 + 32],
                                rhs=whhT_sb[:, k, nb * 512 : (nb + 1) * 512],
                                start=False,
                                stop=(k == KH - 1),
                                tile_position=(0, 32 * j),
                            )
                    np_ = 32 * (gactive[-1] + 1)
                    acts = sc_act.tile([128, 512], f32)
                    nc.scalar.activation(
                        acts[0:np_, 0:384], ps[0:np_, 0:384], AF.Sigmoid
                    )
                    nc.scalar.activation(
                        acts[0:np_, 384:512], ps[0:np_, 384:512], AF.Tanh
                    )
                    cs = c_g[0:np_, w * 128 : (w + 1) * 128]
                    ig = sc_tmp.tile([128, 128], f32)
                    nc.vector.tensor_mul(
                        ig[0:np_], acts[0:np_, 0:128], acts[0:np_, 384:512]
                    )
                    fc = sc_tmp.tile([128, 128], f32)
                    nc.vector.tensor_mul(fc[0:np_], acts[0:np_, 128:256], cs)
                    nc.vector.tensor_add(cs, fc[0:np_], ig[0:np_])
                    tanh_c = sc_tmp.tile([128, 128], f32)
                    nc.scalar.activation(tanh_c[0:np_], cs, AF.Tanh)
                    nc.vector.tensor_mul(
                        h_g[0:np_, w * 128 : (w + 1) * 128],
                        acts[0:np_, 256:384],
                        tanh_c[0:np_],
                    )
                # gather grouped h into natural-layout h (static SBUF->SBUF)
                for j in range(ngroups):
                    nbs = grp_banks[j]
                    if not nbs:
                        continue
                    nc.sync.dma_start(
                        h_sb[:, nbs[0] * 128 : (nbs[-1] + 1) * 128],
                        h_g[32 * j : 32 * j + BL, 0 : len(nbs) * 128],
                    )
                # transposes: 2 column-halves per hidden slice (regular MMs)
                for s in range(KH):
                    pT = sc_psT.tile([128, BL], f32)
                    for m in range(2):
                        nc.tensor.matmul(
                            pT[64 * m : 64 * (m + 1), :],
                            lhsT=h_sb[:, s * 128 + 64 * m : s * 128 + 64 * (m + 1)],
                            rhs=eye8,
                            start=True,
                            stop=True,
                            tile_position=(0, 64 * m),
                        )
                    nc.vector.tensor_copy(hT_next[:, s * 32 : s * 32 + BL], pT)
                nc.sync.dma_start(y_d.ap()[ds(iv * BL, BL), :], h_sb)

            step = step_v1 if scan_mode == "v1" else step_v2

            if use_loop:
                with tc.For_i(
                    0, t_steps, loop_step, hint_engines=(mybir.EngineType.PE,)
                ) as iv:
                    for j in range(loop_step):
                        a, b_ = (hT_a, hT_b) if j % 2 == 0 else (hT_b, hT_a)
                        step(iv + j, a, b_)
            else:
                for t in range(t_steps):
                    a, b_ = (hT_a, hT_b) if t % 2 == 0 else (hT_b, hT_a)
                    step(t, a, b_)

            if scan_mode == "v1":
                nc.sync.dma_start(c_out_d.ap(), c_sb)
            else:
                # c_out from grouped c: per group j, slices are contiguous
                for j in range(ngroups):
                    nbs = grp_banks[j]
                    if not nbs:
                        continue
                    nc.sync.dma_start(
                        c_out_d.ap()[:, nbs[0] * 128 : (nbs[-1] + 1) * 128],
                        c_g[32 * j : 32 * j + BL, 0 : len(nbs) * 128],
                    )

    if split_waits:
        _split_multi_waits(nc, mybir)
    return nc


# ---------------------------------------------------------------------------
# Host-side prep / postprocessing
# ---------------------------------------------------------------------------

def gate_perm():
    """Permutation of the 4H gate axis: per hidden slice s (128 wide),
    blocks ordered [i f o g]. Reference gate order is i,f,g,o."""
    p = []
    src = {0: 0, 1: 1, 2: 3, 3: 2}  # dest block -> ref gate index
    for s in range(H // 128):
        for db in range(4):
            g = src[db]
            p.extend(range(g * H + s * 128, g * H + s * 128 + 128))
    return np.array(p, dtype=np.int64)


def prep_inputs(x, W_emb, b_emb, W_ih, W_hh, b_ih, b_hh):
    perm = gate_perm()
    W_ih_p = np.ascontiguousarray(W_ih[perm])
    W_hh_p = np.ascontiguousarray(W_hh[perm])
    bias_total = (W_ih_p @ b_emb + b_ih[perm] + b_hh[perm]).astype(np.float32)
    shared = {
        "wembT": np.ascontiguousarray(W_emb.T).astype(np.float32),
        "wihT": np.ascontiguousarray(W_ih_p.T).astype(np.float32),
        "whhT": np.ascontiguousarray(W_hh_p.T).astype(np.float32),
        "biasg": bias_total.reshape(1, G4),
    }
    in_maps = []
    for c in range(NCORES):
        xc = np.ascontiguousarray(
            x[c * BL : (c + 1) * BL].reshape(BL * T, R)
        ).astype(np.float32)
        in_maps.append({"x": xc, **shared})
    return in_maps


def assemble_outputs(results):
    outputs = np.empty((B, T, H), np.float32)
    c_t = np.empty((B, H), np.float32)
    for c, r in enumerate(results):
        y = r["y"].reshape(T, BL, H)
        outputs[c * BL : (c + 1) * BL] = y.transpose(1, 0, 2)
        c_t[c * BL : (c + 1) * BL] = r["c_out"]
    h_t = outputs[:, -1, :].copy()
    return outputs, h_t, c_t


_CACHE = {}


def kernel(x, W_emb, b_emb, W_ih, W_hh, b_ih, b_hh):
    from concourse.bass_utils import run_bass_kernel_spmd

    in_maps = prep_inputs(
        np.asarray(x, np.float32),
        np.asarray(W_emb, np.float32),
        np.asarray(b_emb, np.float32),
        np.asarray(W_ih, np.float32),
        np.asarray(W_hh, np.float32),
        np.asarray(b_ih, np.float32),
        np.asarray(b_hh, np.float32),
    )
    if "nc" not in _CACHE:
        _CACHE["nc"] = build_module()
    nc = _CACHE["nc"]
    res = run_bass_kernel_spmd(nc, in_maps, core_ids=list(range(NCORES)))
    return assemble_outputs(res.results)
